# revision 1
# baseline (speedup 1.0000x reference)
"""Multi-head self-attention on 8 Trainium2 NeuronCores (Bass/Tile).

Problem: x[4, 2048, 1024], 16 heads x 64 dim, fused QKV/attention/out-proj.

Sharding (no collectives): core c handles batch b = c//2 and query-half
qh = c%2 (1024 queries), all 16 heads. K/V are computed for the full 2048
tokens of batch b (2x redundancy within a batch pair); outputs are disjoint
[1024, 1024] slices that the host concatenates.

On-chip layout (all fp16 operands, fp32 PSUM accumulation):
  - x^T [1024 in, 2048 tok] per batch, query-half tokens permuted first
  - Q^T/K^T proj: out[feat 128, tok] tiles (feature-major => heads land on
    partitions, d=64), softmax scale 1/8 folded into wq/qb on host
  - scores^T [k 128, q 512] via row-packed head-pair matmuls (d=64 each)
  - exp on ScalarE straight out of PSUM (no max subtraction: |s|/8 <~ 3)
  - P^T accumulated across k-chunks on DVE (fp16), row-sums via ones-matmul,
    reciprocal broadcast back to 128 partitions via a rank-1 matmul
  - PV col-packed per head pair -> A^T [128 feat, q], divided during PSUM
    evacuation
  - out-proj contracts the 8 A^T pair-chunks, bias added on DVE, fp32 out
"""

import numpy as np

EMBED = 1024
NH = 16
D = 64
B = 4
T = 2048
TQ = 1024  # queries per core
NCORES = 8
NIC = EMBED // 128  # 8 contraction chunks
NHP = NH // 2  # 8 head pairs

_PROGRAM = None


def _build_program():
    import concourse.bass as bass
    import concourse.mybir as mybir
    import concourse.tile as tile
    from concourse import bacc

    F16 = mybir.dt.float16
    F32 = mybir.dt.float32
    AF = mybir.ActivationFunctionType

    nc = bacc.Bacc("TRN2", target_bir_lowering=False, debug=False,
                   num_devices=NCORES)

    xT_d = nc.dram_tensor("xT", [EMBED, T], F16, kind="ExternalInput").ap()
    wq_d = nc.dram_tensor("wqT", [EMBED, EMBED], F16, kind="ExternalInput").ap()
    wk_d = nc.dram_tensor("wkT", [EMBED, EMBED], F16, kind="ExternalInput").ap()
    wv_d = nc.dram_tensor("wvT", [EMBED, EMBED], F16, kind="ExternalInput").ap()
    wo_d = nc.dram_tensor("woT", [EMBED, EMBED], F16, kind="ExternalInput").ap()
    qb_d = nc.dram_tensor("qb", [128, NIC], F32, kind="ExternalInput").ap()
    kb_d = nc.dram_tensor("kb", [128, NIC], F32, kind="ExternalInput").ap()
    vbb_d = nc.dram_tensor("vbb", [128, EMBED], F16, kind="ExternalInput").ap()
    obb_d = nc.dram_tensor("obb", [128, EMBED], F32, kind="ExternalInput").ap()
    ones_d = nc.dram_tensor("ones", [128, 1], F16, kind="ExternalInput").ap()
    sel_d = nc.dram_tensor("sel", [1, 256], F16, kind="ExternalInput").ap()
    y_d = nc.dram_tensor("y", [TQ, EMBED], F32, kind="ExternalOutput").ap()

    xT_r = xT_d.rearrange("(c p) t -> c p t", p=128)
    wq_r = wq_d.rearrange("(c p) o -> c p o", p=128)
    wk_r = wk_d.rearrange("(c p) o -> c p o", p=128)
    wv_r = wv_d.rearrange("(c p) o -> c p o", p=128)
    wo_r = wo_d.rearrange("(c p) o -> c p o", p=128)
    y_r = y_d.rearrange("(tb p) o -> tb p o", p=128)

    NKC = T // 128       # 16 key chunks
    NQB = TQ // 512      # 2 query blocks
    NTB = T // 128       # 16 token blocks for V
    NVO = 2              # V out-feature 512-blocks

    with tile.TileContext(nc) as tc:
        from contextlib import ExitStack
        with ExitStack() as ctx:
            cst = ctx.enter_context(tc.tile_pool(name="cst", bufs=1))
            big = ctx.enter_context(tc.tile_pool(name="big", bufs=1))
            wqk = ctx.enter_context(tc.tile_pool(name="wqk", bufs=2))
            qkp = ctx.enter_context(tc.tile_pool(name="qkp", bufs=2))
            pTp = ctx.enter_context(tc.tile_pool(name="pTp", bufs=4))
            accp = ctx.enter_context(tc.tile_pool(name="accp", bufs=2))
            misc = ctx.enter_context(tc.tile_pool(name="misc", bufs=2))
            outp = ctx.enter_context(tc.tile_pool(name="outp", bufs=3))
            ps_st = ctx.enter_context(
                tc.tile_pool(name="ps_st", bufs=2, space="PSUM"))
            ps_pv = ctx.enter_context(
                tc.tile_pool(name="ps_pv", bufs=2, space="PSUM"))
            ps_sm = ctx.enter_context(
                tc.tile_pool(name="ps_sm", bufs=2, space="PSUM"))

            # ---- persistent tiles ----
            xT = big.tile([128, NIC * T], F16, tag="xT")          # 32KB/par
            wv = big.tile([128, NIC * EMBED], F16, tag="wv")      # 16KB
            wo = big.tile([128, NIC * EMBED], F16, tag="wo")      # 16KB
            vv = big.tile([128, NTB * EMBED], F16, tag="vv")      # 32KB
            aT = big.tile([128, NHP * TQ], F16, tag="aT")         # 16KB
            qb_sb = cst.tile([128, NIC], F32, tag="qb")
            kb_sb = cst.tile([128, NIC], F32, tag="kb")
            vbb = cst.tile([128, EMBED], F16, tag="vbb")
            obb = cst.tile([128, EMBED], F32, tag="obb")
            ones = cst.tile([128, 1], F16, tag="ones")
            sel = cst.tile([1, 256], F16, tag="sel")

            nc.sync.dma_start(qb_sb[:], qb_d[:])
            nc.sync.dma_start(kb_sb[:], kb_d[:])
            nc.sync.dma_start(vbb[:], vbb_d[:])
            nc.sync.dma_start(obb[:], obb_d[:])
            nc.sync.dma_start(ones[:], ones_d[:])
            nc.sync.dma_start(sel[:], sel_d[:])
            for c in range(NIC):
                nc.sync.dma_start(xT[:, c * T:(c + 1) * T], xT_r[c])
                nc.sync.dma_start(wv[:, c * EMBED:(c + 1) * EMBED], wv_r[c])
                nc.sync.dma_start(wo[:, c * EMBED:(c + 1) * EMBED], wo_r[c])

            def v_proj(ob):
                # V[tok, feat] for feat block ob*512 : DVE adds bias
                for tb in range(NTB):
                    ps = ps_sm.tile([128, 512], F32, tag="small")
                    for c in range(NIC):
                        nc.tensor.matmul(
                            ps[:],
                            lhsT=xT[:, c * T + tb * 128: c * T + tb * 128 + 128],
                            rhs=wv[:, c * EMBED + ob * 512: c * EMBED + ob * 512 + 512],
                            start=(c == 0), stop=(c == NIC - 1))
                    nc.vector.tensor_add(
                        vv[:, tb * EMBED + ob * 512: tb * EMBED + ob * 512 + 512],
                        ps[:], vbb[:, ob * 512:(ob + 1) * 512])

            v_proj(0)

            for hp in range(NHP):
                if hp == NHP // 2:
                    v_proj(1)

                # ---- load W_q / W_k column-slices for this head pair ----
                wq_sb = wqk.tile([128, NIC * 128], F16, tag="wq")
                wk_sb = wqk.tile([128, NIC * 128], F16, tag="wk")
                for c in range(NIC):
                    nc.sync.dma_start(
                        wq_sb[:, c * 128:(c + 1) * 128],
                        wq_r[c][:, hp * 128:(hp + 1) * 128])
                    nc.sync.dma_start(
                        wk_sb[:, c * 128:(c + 1) * 128],
                        wk_r[c][:, hp * 128:(hp + 1) * 128])

                # ---- K^T pair: [128 feat, 2048 k] ----
                kT = qkp.tile([128, T], F16, tag="kT")
                for tb in range(T // 512):
                    ps = ps_sm.tile([128, 512], F32, tag="small")
                    for c in range(NIC):
                        nc.tensor.matmul(
                            ps[:], lhsT=wk_sb[:, c * 128:(c + 1) * 128],
                            rhs=xT[:, c * T + tb * 512: c * T + tb * 512 + 512],
                            start=(c == 0), stop=(c == NIC - 1))
                    nc.vector.tensor_scalar_add(
                        kT[:, tb * 512:(tb + 1) * 512], ps[:],
                        kb_sb[:, hp:hp + 1])

                # ---- Q^T pair: [128 feat, 1024 q] (first TQ tokens) ----
                qT = qkp.tile([128, TQ], F16, tag="qT")
                for tb in range(TQ // 512):
                    ps = ps_sm.tile([128, 512], F32, tag="small")
                    for c in range(NIC):
                        nc.tensor.matmul(
                            ps[:], lhsT=wq_sb[:, c * 128:(c + 1) * 128],
                            rhs=xT[:, c * T + tb * 512: c * T + tb * 512 + 512],
                            start=(c == 0), stop=(c == NIC - 1))
                    nc.vector.tensor_scalar_add(
                        qT[:, tb * 512:(tb + 1) * 512], ps[:],
                        qb_sb[:, hp:hp + 1])

                # ---- attention for this head pair ----
                for qb in range(NQB):
                    pv = ps_pv.tile([128, 512], F32, tag="pv")
                    acc = accp.tile([128, 1024], F16, tag="acc")
                    prev_pT = None
                    for kc in range(NKC):
                        st = ps_st.tile([128, 1024], F32, tag="st")
                        nc.tensor.matmul(
                            st[:, 0:512],
                            lhsT=kT[0:64, kc * 128:(kc + 1) * 128],
                            rhs=qT[0:64, qb * 512:(qb + 1) * 512],
                            start=True, stop=True)
                        nc.tensor.matmul(
                            st[:, 512:1024],
                            lhsT=kT[64:128, kc * 128:(kc + 1) * 128],
                            rhs=qT[64:128, qb * 512:(qb + 1) * 512],
                            start=True, stop=True, tile_position=(64, 0))
                        pT = pTp.tile([128, 1024], F16, tag="pT")
                        nc.scalar.activation(pT[:], st[:], AF.Exp)
                        with nc.allow_low_precision(
                                reason="fp16 softmax partial-sum accumulate"):
                            if kc == 1:
                                nc.vector.tensor_add(
                                    acc[:], prev_pT[:], pT[:])
                            elif kc > 1:
                                nc.vector.tensor_add(acc[:], acc[:], pT[:])
                        prev_pT = pT
                        nc.tensor.matmul(
                            pv[0:64, :],
                            lhsT=vv[:, kc * EMBED + hp * 128:
                                    kc * EMBED + hp * 128 + 64],
                            rhs=pT[:, 0:512],
                            start=(kc == 0), stop=(kc == NKC - 1))
                        nc.tensor.matmul(
                            pv[64:128, :],
                            lhsT=vv[:, kc * EMBED + hp * 128 + 64:
                                    kc * EMBED + hp * 128 + 128],
                            rhs=pT[:, 512:1024],
                            start=(kc == 0), stop=(kc == NKC - 1),
                            tile_position=(0, 64))

                    # softmax denominators: ones^T @ acc -> [1, 512] per head
                    sums = ps_sm.tile([128, 512], F32, tag="small")
                    nc.tensor.matmul(sums[0:1, :], lhsT=ones[:],
                                     rhs=acc[:, 0:512], start=True, stop=True)
                    nc.tensor.matmul(sums[32:33, :], lhsT=ones[:],
                                     rhs=acc[:, 512:1024], start=True,
                                     stop=True, tile_position=(0, 32))
                    recip = misc.tile([1, 1024], F16, tag="recip")
                    with nc.allow_low_precision(
                            reason="fp16 softmax reciprocal"):
                        nc.vector.reciprocal(recip[:, 0:512], sums[0:1, :])
                        nc.vector.reciprocal(recip[:, 512:1024],
                                             sums[32:33, :])
                    bc = ps_sm.tile([128, 512], F32, tag="small")
                    nc.tensor.matmul(bc[:], lhsT=sel[:, 0:128],
                                     rhs=recip[:, 0:512], start=True,
                                     stop=False)
                    nc.tensor.matmul(bc[:], lhsT=sel[:, 128:256],
                                     rhs=recip[:, 512:1024], start=False,
                                     stop=True)
                    bc_sb = misc.tile([128, 512], F32, tag="bc_sb")
                    nc.vector.tensor_copy(bc_sb[:], bc[:])
                    nc.vector.tensor_mul(
                        aT[:, hp * TQ + qb * 512: hp * TQ + qb * 512 + 512],
                        pv[:], bc_sb[:])

            # ---- out projection: y[tok, feat] ----
            for tb in range(TQ // 128):
                for ob in range(2):
                    ps = ps_sm.tile([128, 512], F32, tag="small")
                    for hp in range(NHP):
                        nc.tensor.matmul(
                            ps[:],
                            lhsT=aT[:, hp * TQ + tb * 128:
                                    hp * TQ + tb * 128 + 128],
                            rhs=wo[:, hp * EMBED + ob * 512:
                                   hp * EMBED + ob * 512 + 512],
                            start=(hp == 0), stop=(hp == NHP - 1))
                    out_sb = outp.tile([128, 512], F32, tag="out")
                    nc.vector.tensor_add(out_sb[:], ps[:],
                                         obb[:, ob * 512:(ob + 1) * 512])
                    nc.sync.dma_start(y_r[tb][:, ob * 512:(ob + 1) * 512],
                                      out_sb[:])

    nc.compile()
    return nc


def _get_program():
    global _PROGRAM
    if _PROGRAM is None:
        _PROGRAM = _build_program()
    return _PROGRAM


def _make_in_maps(x, q_w, q_b, k_w, k_b, v_w, v_b, o_w, o_b):
    f16 = np.float16
    # softmax scale folded into the Q projection
    wqT = np.ascontiguousarray((q_w.astype(np.float32).T / 8.0)).astype(f16)
    wkT = np.ascontiguousarray(k_w.astype(np.float32).T).astype(f16)
    wvT = np.ascontiguousarray(v_w.astype(np.float32).T).astype(f16)
    woT = np.ascontiguousarray(o_w.astype(np.float32).T).astype(f16)
    qb = np.ascontiguousarray(
        (q_b.astype(np.float32) / 8.0).reshape(NIC, 128).T)
    kb = np.ascontiguousarray(k_b.astype(np.float32).reshape(NIC, 128).T)
    vbb = np.broadcast_to(v_b.astype(np.float32), (128, EMBED)).astype(f16)
    vbb = np.ascontiguousarray(vbb)
    obb = np.ascontiguousarray(
        np.broadcast_to(o_b.astype(np.float32), (128, EMBED)))
    ones = np.ones((128, 1), f16)
    sel = np.zeros((1, 256), f16)
    sel[0, 0:64] = 1.0
    sel[0, 192:256] = 1.0
    in_maps = []
    for c in range(NCORES):
        b, qh = c // 2, c % 2
        xb = x[b].astype(np.float32)  # [T, EMBED]
        if qh == 0:
            xp = xb
        else:
            # query half first; K/V order is irrelevant (softmax sums over k)
            xp = np.concatenate([xb[TQ:], xb[:TQ]], axis=0)
        xT = np.ascontiguousarray(xp.T).astype(f16)
        in_maps.append({
            "xT": xT, "wqT": wqT, "wkT": wkT, "wvT": wvT, "woT": woT,
            "qb": qb, "kb": kb, "vbb": vbb, "obb": obb,
            "ones": ones, "sel": sel,
        })
    return in_maps


def kernel(x, mask, q_w, q_b, k_w, k_b, v_w, v_b, o_w, o_b):
    from concourse.bass_utils import run_bass_kernel_spmd

    nc = _get_program()
    x = np.asarray(x)
    in_maps = _make_in_maps(np.asarray(x), np.asarray(q_w), np.asarray(q_b),
                            np.asarray(k_w), np.asarray(k_b),
                            np.asarray(v_w), np.asarray(v_b),
                            np.asarray(o_w), np.asarray(o_b))
    res = run_bass_kernel_spmd(nc, in_maps, list(range(NCORES)))
    out = np.empty((B, T, EMBED), np.float32)
    for c in range(NCORES):
        b, qh = c // 2, c % 2
        out[b, qh * TQ:(qh + 1) * TQ, :] = res.results[c]["y"]
    return out


# revision 6
# speedup vs baseline: 1.1786x; 1.1786x over previous
"""Multi-head self-attention on 8 Trainium2 NeuronCores (Bass/Tile).

Problem: x[4, 2048, 1024], 16 heads x 64 dim, fused QKV/attention/out-proj.

Sharding (no collectives): core c handles batch b = c//2 and query-half
qh = c%2 (1024 queries), all 16 heads. K/V are computed for the full 2048
tokens of batch b (2x redundancy within a batch pair); outputs are disjoint
[1024, 1024] slices that the host concatenates.

On-chip layout (all fp16 operands, fp32 PSUM accumulation):
  - x^T [1024 in, 2048 tok] per batch, query-half tokens permuted first
  - Q^T/K^T proj: out[feat 128, tok] tiles (feature-major => heads land on
    partitions, d=64), softmax scale 1/8 folded into wq/qb on host
  - scores^T [k 128, q 512] via row-packed head-pair matmuls (d=64 each)
  - exp on ScalarE straight out of PSUM (no max subtraction: |s|/8 <~ 3)
  - P^T accumulated across k-chunks on DVE (fp16), row-sums via ones-matmul,
    reciprocal broadcast back to 128 partitions via a rank-1 matmul
  - PV col-packed per head pair -> A^T [128 feat, q], divided during PSUM
    evacuation
  - out-proj contracts the 8 A^T pair-chunks, bias added on DVE, fp32 out
"""

import numpy as np

EMBED = 1024
NH = 16
D = 64
B = 4
T = 2048
TQ = 1024  # queries per core
NCORES = 8
NIC = EMBED // 128  # 8 contraction chunks
NHP = NH // 2  # 8 head pairs

_PROGRAM = None


def _build_program():
    import concourse.bass as bass
    import concourse.mybir as mybir
    import concourse.tile as tile
    from concourse import bacc

    F16 = mybir.dt.float16
    F32 = mybir.dt.float32
    AF = mybir.ActivationFunctionType

    nc = bacc.Bacc("TRN2", target_bir_lowering=False, debug=False,
                   num_devices=NCORES)

    xT_d = nc.dram_tensor("xT", [EMBED, T], F16, kind="ExternalInput").ap()
    wq_d = nc.dram_tensor("wqT", [EMBED, EMBED], F16, kind="ExternalInput").ap()
    wk_d = nc.dram_tensor("wkT", [EMBED, EMBED], F16, kind="ExternalInput").ap()
    wv_d = nc.dram_tensor("wvT", [EMBED, EMBED], F16, kind="ExternalInput").ap()
    wo_d = nc.dram_tensor("woT", [EMBED, EMBED], F16, kind="ExternalInput").ap()
    qb_d = nc.dram_tensor("qb", [128, NIC], F32, kind="ExternalInput").ap()
    kb_d = nc.dram_tensor("kb", [128, NIC], F32, kind="ExternalInput").ap()
    vbb_d = nc.dram_tensor("vbb", [128, EMBED], F16, kind="ExternalInput").ap()
    obb_d = nc.dram_tensor("obb", [128, EMBED], F32, kind="ExternalInput").ap()
    ones_d = nc.dram_tensor("ones", [128, 1], F16, kind="ExternalInput").ap()
    sel_d = nc.dram_tensor("sel", [1, 256], F16, kind="ExternalInput").ap()
    y_d = nc.dram_tensor("y", [TQ, EMBED], F32, kind="ExternalOutput").ap()

    xT_r = xT_d.rearrange("(c p) t -> c p t", p=128)
    wq_r = wq_d.rearrange("(c p) o -> c p o", p=128)
    wk_r = wk_d.rearrange("(c p) o -> c p o", p=128)
    wv_r = wv_d.rearrange("(c p) o -> c p o", p=128)
    wo_r = wo_d.rearrange("(c p) o -> c p o", p=128)
    y_r = y_d.rearrange("(tb p) o -> tb p o", p=128)

    NKC = T // 128       # 16 key chunks
    NQB = TQ // 512      # 2 query blocks
    NTB = T // 128       # 16 token blocks for V
    NVO = 2              # V out-feature 512-blocks

    with tile.TileContext(nc) as tc:
        from contextlib import ExitStack
        with ExitStack() as ctx:
            cst = ctx.enter_context(tc.tile_pool(name="cst", bufs=1))
            big = ctx.enter_context(tc.tile_pool(name="big", bufs=1))
            wqk = ctx.enter_context(tc.tile_pool(name="wqk", bufs=2))
            qkp = ctx.enter_context(tc.tile_pool(name="qkp", bufs=2))
            pTp = ctx.enter_context(tc.tile_pool(name="pTp", bufs=4))
            accp = ctx.enter_context(tc.tile_pool(name="accp", bufs=2))
            misc = ctx.enter_context(tc.tile_pool(name="misc", bufs=2))
            outp = ctx.enter_context(tc.tile_pool(name="outp", bufs=3))
            ps_st = ctx.enter_context(
                tc.tile_pool(name="ps_st", bufs=2, space="PSUM"))
            ps_pv = ctx.enter_context(
                tc.tile_pool(name="ps_pv", bufs=2, space="PSUM"))
            ps_sm = ctx.enter_context(
                tc.tile_pool(name="ps_sm", bufs=2, space="PSUM"))

            # ---- persistent tiles ----
            xT = big.tile([128, NIC * T], F16, tag="xT")          # 32KB/par
            wv = big.tile([128, NIC * EMBED], F16, tag="wv")      # 16KB
            wo = big.tile([128, NIC * EMBED], F16, tag="wo")      # 16KB
            vv = big.tile([128, NTB * EMBED], F16, tag="vv")      # 32KB
            aT = big.tile([128, NHP * TQ], F16, tag="aT")         # 16KB
            qb_sb = cst.tile([128, NIC], F32, tag="qb")
            kb_sb = cst.tile([128, NIC], F32, tag="kb")
            vbb = cst.tile([128, EMBED], F16, tag="vbb")
            obb = cst.tile([128, EMBED], F32, tag="obb")
            ones = cst.tile([128, 1], F16, tag="ones")
            sel = cst.tile([1, 256], F16, tag="sel")

            nc.sync.dma_start(qb_sb[:], qb_d[:])
            nc.sync.dma_start(kb_sb[:], kb_d[:])
            nc.sync.dma_start(vbb[:], vbb_d[:])
            nc.sync.dma_start(obb[:], obb_d[:])
            nc.sync.dma_start(ones[:], ones_d[:])
            nc.sync.dma_start(sel[:], sel_d[:])
            for c in range(NIC):
                nc.sync.dma_start(xT[:, c * T:(c + 1) * T], xT_r[c])
                nc.sync.dma_start(wv[:, c * EMBED:(c + 1) * EMBED], wv_r[c])
                nc.sync.dma_start(wo[:, c * EMBED:(c + 1) * EMBED], wo_r[c])

            def v_proj_tb(ob, tb):
                # V[tok, feat] for token block tb, feat block ob*512
                ps = ps_sm.tile([128, 512], F32, tag="small")
                for c in range(NIC):
                    nc.tensor.matmul(
                        ps[:],
                        lhsT=xT[:, c * T + tb * 128: c * T + tb * 128 + 128],
                        rhs=wv[:, c * EMBED + ob * 512: c * EMBED + ob * 512 + 512],
                        start=(c == 0), stop=(c == NIC - 1))
                nc.vector.tensor_add(
                    vv[:, tb * EMBED + ob * 512: tb * EMBED + ob * 512 + 512],
                    ps[:], vbb[:, ob * 512:(ob + 1) * 512])

            for hp in range(NHP):
                # V feature-block ob is produced just-in-time, interleaved
                # with the first attention loop that consumes it (hp 0 / 4)
                v_ob = hp // (NHP // 2)
                v_interleave = hp % (NHP // 2) == 0

                # ---- load W_q / W_k column-slices for this head pair ----
                wq_sb = wqk.tile([128, NIC * 128], F16, tag="wq")
                wk_sb = wqk.tile([128, NIC * 128], F16, tag="wk")
                for c in range(NIC):
                    nc.sync.dma_start(
                        wq_sb[:, c * 128:(c + 1) * 128],
                        wq_r[c][:, hp * 128:(hp + 1) * 128])
                    nc.sync.dma_start(
                        wk_sb[:, c * 128:(c + 1) * 128],
                        wk_r[c][:, hp * 128:(hp + 1) * 128])

                # ---- K^T pair: [128 feat, 2048 k] ----
                kT = qkp.tile([128, T], F16, tag="kT")
                for tb in range(T // 512):
                    ps = ps_sm.tile([128, 512], F32, tag="small")
                    for c in range(NIC):
                        nc.tensor.matmul(
                            ps[:], lhsT=wk_sb[:, c * 128:(c + 1) * 128],
                            rhs=xT[:, c * T + tb * 512: c * T + tb * 512 + 512],
                            start=(c == 0), stop=(c == NIC - 1))
                    nc.vector.tensor_scalar_add(
                        kT[:, tb * 512:(tb + 1) * 512], ps[:],
                        kb_sb[:, hp:hp + 1])

                # ---- Q^T pair: [128 feat, 1024 q] (first TQ tokens) ----
                qT = qkp.tile([128, TQ], F16, tag="qT")
                for tb in range(TQ // 512):
                    ps = ps_sm.tile([128, 512], F32, tag="small")
                    for c in range(NIC):
                        nc.tensor.matmul(
                            ps[:], lhsT=wq_sb[:, c * 128:(c + 1) * 128],
                            rhs=xT[:, c * T + tb * 512: c * T + tb * 512 + 512],
                            start=(c == 0), stop=(c == NIC - 1))
                    nc.vector.tensor_scalar_add(
                        qT[:, tb * 512:(tb + 1) * 512], ps[:],
                        qb_sb[:, hp:hp + 1])

                # ---- attention for this head pair ----
                for qb in range(NQB):
                    pv = ps_pv.tile([128, 512], F32, tag="pv")
                    acc = accp.tile([128, 1024], F16, tag="acc")
                    prev_pT = None
                    for kc in range(NKC):
                        if v_interleave and qb == 0:
                            v_proj_tb(v_ob, kc)
                        st = ps_st.tile([128, 1024], F32, tag="st")
                        nc.tensor.matmul(
                            st[:, 0:512],
                            lhsT=kT[0:64, kc * 128:(kc + 1) * 128],
                            rhs=qT[0:64, qb * 512:(qb + 1) * 512],
                            start=True, stop=True)
                        nc.tensor.matmul(
                            st[:, 512:1024],
                            lhsT=kT[64:128, kc * 128:(kc + 1) * 128],
                            rhs=qT[64:128, qb * 512:(qb + 1) * 512],
                            start=True, stop=True, tile_position=(64, 0))
                        pT = pTp.tile([128, 1024], F16, tag="pT")
                        nc.scalar.activation(pT[:], st[:], AF.Exp)
                        with nc.allow_low_precision(
                                reason="fp16 softmax partial-sum accumulate"):
                            if kc == 1:
                                nc.vector.tensor_add(
                                    acc[:], prev_pT[:], pT[:])
                            elif kc > 1:
                                nc.vector.tensor_add(acc[:], acc[:], pT[:])
                        prev_pT = pT
                        nc.tensor.matmul(
                            pv[0:64, :],
                            lhsT=vv[:, kc * EMBED + hp * 128:
                                    kc * EMBED + hp * 128 + 64],
                            rhs=pT[:, 0:512],
                            start=(kc == 0), stop=(kc == NKC - 1))
                        nc.tensor.matmul(
                            pv[64:128, :],
                            lhsT=vv[:, kc * EMBED + hp * 128 + 64:
                                    kc * EMBED + hp * 128 + 128],
                            rhs=pT[:, 512:1024],
                            start=(kc == 0), stop=(kc == NKC - 1),
                            tile_position=(0, 64))

                    # softmax denominators: ones^T @ acc -> [1, 512] per head
                    sums = ps_sm.tile([128, 512], F32, tag="small")
                    nc.tensor.matmul(sums[0:1, :], lhsT=ones[:],
                                     rhs=acc[:, 0:512], start=True, stop=True)
                    nc.tensor.matmul(sums[32:33, :], lhsT=ones[:],
                                     rhs=acc[:, 512:1024], start=True,
                                     stop=True, tile_position=(0, 32))
                    # copy the two sum-rows (partitions 0 and 32) to SBUF in
                    # one strided DVE op, broadcast raw sums to 128
                    # partitions with a rank-1 matmul, then one fast
                    # reciprocal over the broadcast tile
                    sums_sb = misc.tile([1, 1024], F16, tag="sums_sb")
                    with nc.allow_low_precision(
                            reason="softmax denominators, fp16 ample"):
                        nc.vector.tensor_copy(
                            sums_sb[:, 0:512], sums[0:1, 0:512])
                        nc.vector.tensor_copy(
                            sums_sb[:, 512:1024], sums[32:33, 0:512])
                    bc = ps_sm.tile([128, 512], F32, tag="small")
                    nc.tensor.matmul(bc[:], lhsT=sel[:, 0:128],
                                     rhs=sums_sb[:, 0:512], start=True,
                                     stop=False)
                    nc.tensor.matmul(bc[:], lhsT=sel[:, 128:256],
                                     rhs=sums_sb[:, 512:1024], start=False,
                                     stop=True)
                    bc_sb = misc.tile([128, 512], F32, tag="bc_sb")
                    nc.vector.reciprocal_approx_fast(bc_sb[:], bc[:])
                    nc.vector.tensor_mul(
                        aT[:, hp * TQ + qb * 512: hp * TQ + qb * 512 + 512],
                        pv[:], bc_sb[:])

            # ---- out projection: y[tok, feat] ----
            for tb in range(TQ // 128):
                for ob in range(2):
                    ps = ps_sm.tile([128, 512], F32, tag="small")
                    for hp in range(NHP):
                        nc.tensor.matmul(
                            ps[:],
                            lhsT=aT[:, hp * TQ + tb * 128:
                                    hp * TQ + tb * 128 + 128],
                            rhs=wo[:, hp * EMBED + ob * 512:
                                   hp * EMBED + ob * 512 + 512],
                            start=(hp == 0), stop=(hp == NHP - 1))
                    out_sb = outp.tile([128, 512], F32, tag="out")
                    nc.vector.tensor_add(out_sb[:], ps[:],
                                         obb[:, ob * 512:(ob + 1) * 512])
                    nc.sync.dma_start(y_r[tb][:, ob * 512:(ob + 1) * 512],
                                      out_sb[:])

    nc.compile()
    return nc


def _get_program():
    global _PROGRAM
    if _PROGRAM is None:
        _PROGRAM = _build_program()
    return _PROGRAM


def _make_in_maps(x, q_w, q_b, k_w, k_b, v_w, v_b, o_w, o_b):
    f16 = np.float16
    # softmax scale folded into the Q projection
    wqT = np.ascontiguousarray((q_w.astype(np.float32).T / 8.0)).astype(f16)
    wkT = np.ascontiguousarray(k_w.astype(np.float32).T).astype(f16)
    wvT = np.ascontiguousarray(v_w.astype(np.float32).T).astype(f16)
    woT = np.ascontiguousarray(o_w.astype(np.float32).T).astype(f16)
    qb = np.ascontiguousarray(
        (q_b.astype(np.float32) / 8.0).reshape(NIC, 128).T)
    kb = np.ascontiguousarray(k_b.astype(np.float32).reshape(NIC, 128).T)
    vbb = np.broadcast_to(v_b.astype(np.float32), (128, EMBED)).astype(f16)
    vbb = np.ascontiguousarray(vbb)
    obb = np.ascontiguousarray(
        np.broadcast_to(o_b.astype(np.float32), (128, EMBED)))
    ones = np.ones((128, 1), f16)
    sel = np.zeros((1, 256), f16)
    sel[0, 0:64] = 1.0
    sel[0, 192:256] = 1.0
    in_maps = []
    for c in range(NCORES):
        b, qh = c // 2, c % 2
        xb = x[b].astype(np.float32)  # [T, EMBED]
        if qh == 0:
            xp = xb
        else:
            # query half first; K/V order is irrelevant (softmax sums over k)
            xp = np.concatenate([xb[TQ:], xb[:TQ]], axis=0)
        xT = np.ascontiguousarray(xp.T).astype(f16)
        in_maps.append({
            "xT": xT, "wqT": wqT, "wkT": wkT, "wvT": wvT, "woT": woT,
            "qb": qb, "kb": kb, "vbb": vbb, "obb": obb,
            "ones": ones, "sel": sel,
        })
    return in_maps


def kernel(x, mask, q_w, q_b, k_w, k_b, v_w, v_b, o_w, o_b):
    from concourse.bass_utils import run_bass_kernel_spmd

    nc = _get_program()
    x = np.asarray(x)
    in_maps = _make_in_maps(np.asarray(x), np.asarray(q_w), np.asarray(q_b),
                            np.asarray(k_w), np.asarray(k_b),
                            np.asarray(v_w), np.asarray(v_b),
                            np.asarray(o_w), np.asarray(o_b))
    res = run_bass_kernel_spmd(nc, in_maps, list(range(NCORES)))
    out = np.empty((B, T, EMBED), np.float32)
    for c in range(NCORES):
        b, qh = c // 2, c % 2
        out[b, qh * TQ:(qh + 1) * TQ, :] = res.results[c]["y"]
    return out


# revision 11
# speedup vs baseline: 1.3177x; 1.1180x over previous
"""Multi-head self-attention on 8 Trainium2 NeuronCores (Bass/Tile).

Problem: x[4, 2048, 1024], 16 heads x 64 dim, fused QKV/attention/out-proj.

Sharding (no collectives): core c handles batch b = c//2 and query-half
qh = c%2 (1024 queries), all 16 heads. K/V are computed for the full 2048
tokens of batch b (2x redundancy within a batch pair); outputs are disjoint
[1024, 1024] slices that the host concatenates.

On-chip layout (all fp16 operands, fp32 PSUM accumulation):
  - x^T [1024 in, 2048 tok] per batch, query-half tokens permuted first
  - Q^T/K^T proj: out[feat 128, tok] tiles (feature-major => heads land on
    partitions, d=64), softmax scale 1/8 folded into wq/qb on host
  - scores^T [k 128, q 512] via row-packed head-pair matmuls (d=64 each)
  - exp on ScalarE straight out of PSUM (no max subtraction: |s|/8 <~ 3)
  - P^T accumulated across k-chunks on DVE (fp16), row-sums via ones-matmul,
    reciprocal broadcast back to 128 partitions via a rank-1 matmul
  - PV col-packed per head pair -> A^T [128 feat, q], divided during PSUM
    evacuation
  - out-proj contracts the 8 A^T pair-chunks, bias added on DVE, fp32 out
"""

import numpy as np

EMBED = 1024
NH = 16
D = 64
B = 4
T = 2048
TQ = 1024  # queries per core
NCORES = 8
NIC = EMBED // 128  # 8 contraction chunks
NHP = NH // 2  # 8 head pairs

_PROGRAM = None


def _build_program():
    import concourse.bass as bass
    import concourse.mybir as mybir
    import concourse.tile as tile
    from concourse import bacc

    F16 = mybir.dt.float16
    F32 = mybir.dt.float32
    AF = mybir.ActivationFunctionType

    nc = bacc.Bacc("TRN2", target_bir_lowering=False, debug=False,
                   num_devices=NCORES)

    xT_d = nc.dram_tensor("xT", [EMBED, T], F16, kind="ExternalInput").ap()
    wq_d = nc.dram_tensor("wqT", [EMBED, EMBED], F16, kind="ExternalInput").ap()
    wk_d = nc.dram_tensor("wkT", [EMBED, EMBED], F16, kind="ExternalInput").ap()
    wv_d = nc.dram_tensor("wvT", [EMBED, EMBED], F16, kind="ExternalInput").ap()
    wo_d = nc.dram_tensor("woT", [EMBED, EMBED], F16, kind="ExternalInput").ap()
    qb_d = nc.dram_tensor("qb", [128, NIC], F32, kind="ExternalInput").ap()
    kb_d = nc.dram_tensor("kb", [128, NIC], F32, kind="ExternalInput").ap()
    vbb_d = nc.dram_tensor("vbb", [128, EMBED], F16, kind="ExternalInput").ap()
    obb_d = nc.dram_tensor("obb", [128, EMBED], F32, kind="ExternalInput").ap()
    ones_d = nc.dram_tensor("ones", [128, 1], F16, kind="ExternalInput").ap()
    sel_d = nc.dram_tensor("sel", [1, 256], F16, kind="ExternalInput").ap()
    y_d = nc.dram_tensor("y", [TQ, EMBED], F32, kind="ExternalOutput").ap()

    xT_r = xT_d.rearrange("(c p) t -> c p t", p=128)
    wq_r = wq_d.rearrange("(c p) o -> c p o", p=128)
    wk_r = wk_d.rearrange("(c p) o -> c p o", p=128)
    wv_r = wv_d.rearrange("(c p) o -> c p o", p=128)
    wo_r = wo_d.rearrange("(c p) o -> c p o", p=128)
    y_r = y_d.rearrange("(tb p) o -> tb p o", p=128)

    NKC = T // 128       # 16 key chunks
    NQB = TQ // 512      # 2 query blocks
    NTB = T // 128       # 16 token blocks for V
    NVO = 2              # V out-feature 512-blocks

    with tile.TileContext(nc) as tc:
        from contextlib import ExitStack
        with ExitStack() as ctx:
            cst = ctx.enter_context(tc.tile_pool(name="cst", bufs=1))
            big = ctx.enter_context(tc.tile_pool(name="big", bufs=1))
            wqk = ctx.enter_context(tc.tile_pool(name="wqk", bufs=2))
            qkp = ctx.enter_context(tc.tile_pool(name="qkp", bufs=2))
            pTp = ctx.enter_context(tc.tile_pool(name="pTp", bufs=4))
            accp = ctx.enter_context(tc.tile_pool(name="accp", bufs=2))
            misc = ctx.enter_context(tc.tile_pool(name="misc", bufs=2))
            outp = ctx.enter_context(tc.tile_pool(name="outp", bufs=3))
            ps_st = ctx.enter_context(
                tc.tile_pool(name="ps_st", bufs=2, space="PSUM"))
            ps_pv = ctx.enter_context(
                tc.tile_pool(name="ps_pv", bufs=2, space="PSUM"))
            ps_sm = ctx.enter_context(
                tc.tile_pool(name="ps_sm", bufs=2, space="PSUM"))

            # ---- persistent tiles ----
            xT = big.tile([128, NIC * T], F16, tag="xT")          # 32KB/par
            wv = big.tile([128, NIC * EMBED], F16, tag="wv")      # 16KB
            wo = big.tile([128, NIC * EMBED], F16, tag="wo")      # 16KB
            vv = big.tile([128, NTB * EMBED], F16, tag="vv")      # 32KB
            aT = big.tile([128, NHP * TQ], F16, tag="aT")         # 16KB
            qb_sb = cst.tile([128, NIC], F32, tag="qb")
            kb_sb = cst.tile([128, NIC], F32, tag="kb")
            vbb = cst.tile([128, EMBED], F16, tag="vbb")
            obb = cst.tile([128, EMBED], F32, tag="obb")
            ones = cst.tile([128, 1], F16, tag="ones")
            sel = cst.tile([1, 256], F16, tag="sel")

            nc.sync.dma_start(qb_sb[:], qb_d[:])
            nc.sync.dma_start(kb_sb[:], kb_d[:])
            nc.sync.dma_start(vbb[:], vbb_d[:])
            nc.sync.dma_start(obb[:], obb_d[:])
            nc.sync.dma_start(ones[:], ones_d[:])
            nc.sync.dma_start(sel[:], sel_d[:])
            for c in range(NIC):
                nc.sync.dma_start(xT[:, c * T:(c + 1) * T], xT_r[c])
                nc.sync.dma_start(wv[:, c * EMBED:(c + 1) * EMBED], wv_r[c])

            def v_proj_tb(ob, tb):
                # V[tok, feat] for token block tb, feat block ob*512
                ps = ps_sm.tile([128, 512], F32, tag="small")
                for c in range(NIC):
                    nc.tensor.matmul(
                        ps[:],
                        lhsT=xT[:, c * T + tb * 128: c * T + tb * 128 + 128],
                        rhs=wv[:, c * EMBED + ob * 512: c * EMBED + ob * 512 + 512],
                        start=(c == 0), stop=(c == NIC - 1))
                nc.vector.tensor_add(
                    vv[:, tb * EMBED + ob * 512: tb * EMBED + ob * 512 + 512],
                    ps[:], vbb[:, ob * 512:(ob + 1) * 512])

            # ---- per-head-pair K/Q projection, emitted one pair AHEAD,
            # interleaved into the previous pair's attention loop so the
            # scalar engine never drains between pairs ----
            kq = {}

            def alloc_kq(hp):
                wq_sb = wqk.tile([128, NIC * 128], F16, tag="wq")
                wk_sb = wqk.tile([128, NIC * 128], F16, tag="wk")
                for c in range(NIC):
                    nc.sync.dma_start(
                        wq_sb[:, c * 128:(c + 1) * 128],
                        wq_r[c][:, hp * 128:(hp + 1) * 128])
                    nc.sync.dma_start(
                        wk_sb[:, c * 128:(c + 1) * 128],
                        wk_r[c][:, hp * 128:(hp + 1) * 128])
                kT = qkp.tile([128, T], F16, tag="kT")
                qT = qkp.tile([128, TQ], F16, tag="qT")
                kq[hp] = (wq_sb, wk_sb, kT, qT)

            def k_proj_tb(hp, tb):
                wq_sb, wk_sb, kT, qT = kq[hp]
                ps = ps_sm.tile([128, 512], F32, tag="small")
                for c in range(NIC):
                    nc.tensor.matmul(
                        ps[:], lhsT=wk_sb[:, c * 128:(c + 1) * 128],
                        rhs=xT[:, c * T + tb * 512: c * T + tb * 512 + 512],
                        start=(c == 0), stop=(c == NIC - 1))
                nc.vector.tensor_scalar_add(
                    kT[:, tb * 512:(tb + 1) * 512], ps[:], kb_sb[:, hp:hp + 1])

            def q_proj_tb(hp, tb):
                wq_sb, wk_sb, kT, qT = kq[hp]
                ps = ps_sm.tile([128, 512], F32, tag="small")
                for c in range(NIC):
                    nc.tensor.matmul(
                        ps[:], lhsT=wq_sb[:, c * 128:(c + 1) * 128],
                        rhs=xT[:, c * T + tb * 512: c * T + tb * 512 + 512],
                        start=(c == 0), stop=(c == NIC - 1))
                nc.vector.tensor_scalar_add(
                    qT[:, tb * 512:(tb + 1) * 512], ps[:], qb_sb[:, hp:hp + 1])

            def o_proj_unit(tb, ob):
                ps = ps_sm.tile([128, 512], F32, tag="small")
                for f in range(NHP):
                    nc.tensor.matmul(
                        ps[:],
                        lhsT=aT[:, f * TQ + tb * 128: f * TQ + tb * 128 + 128],
                        rhs=wo[:, f * EMBED + ob * 512:
                               f * EMBED + ob * 512 + 512],
                        start=(f == 0), stop=(f == NHP - 1))
                out_sb = outp.tile([128, 512], F32, tag="out")
                nc.vector.tensor_add(out_sb[:], ps[:],
                                     obb[:, ob * 512:(ob + 1) * 512])
                nc.sync.dma_start(y_r[tb][:, ob * 512:(ob + 1) * 512],
                                  out_sb[:])

            alloc_kq(0)
            for tb in range(T // 512):
                k_proj_tb(0, tb)
            for tb in range(TQ // 512):
                q_proj_tb(0, tb)
            for c in range(NIC):
                nc.sync.dma_start(wo[:, c * EMBED:(c + 1) * EMBED], wo_r[c])

            for hp in range(NHP):
                # V feature-block ob produced just-in-time, interleaved with
                # the first attention loop that consumes it (hp 0 / 4)
                v_ob = hp // (NHP // 2)
                v_interleave = hp % (NHP // 2) == 0
                _, _, kT, qT = kq[hp]

                for qb in range(NQB):
                    if hp + 1 < NHP and qb == (1 if v_interleave else 0):
                        alloc_kq(hp + 1)
                    # interleave units: (kc -> emit) for next-pair proj and
                    # the first half of the out-projection under hp7 qb1
                    units = {}
                    if hp + 1 < NHP:
                        if v_interleave:
                            if qb == 1:
                                units = {0: (k_proj_tb, hp + 1, 0),
                                         4: (k_proj_tb, hp + 1, 1),
                                         8: (k_proj_tb, hp + 1, 2),
                                         12: (k_proj_tb, hp + 1, 3),
                                         2: (q_proj_tb, hp + 1, 0),
                                         10: (q_proj_tb, hp + 1, 1)}
                        else:
                            if qb == 0:
                                units = {4: (k_proj_tb, hp + 1, 0),
                                         12: (k_proj_tb, hp + 1, 1)}
                            else:
                                units = {4: (k_proj_tb, hp + 1, 2),
                                         12: (k_proj_tb, hp + 1, 3),
                                         2: (q_proj_tb, hp + 1, 0),
                                         10: (q_proj_tb, hp + 1, 1)}
                    elif qb == 1:
                        units = {2 * u + 1: (o_proj_unit, u // 2, u % 2)
                                 for u in range(8)}
                    pv = ps_pv.tile([128, 512], F32, tag="pv")
                    acc = accp.tile([128, 1024], F16, tag="acc")
                    prev_pT = None
                    for kc in range(NKC):
                        if v_interleave and qb == 0:
                            v_proj_tb(v_ob, kc)
                        if kc in units:
                            fn, a0, a1 = units[kc]
                            fn(a0, a1)
                        st = ps_st.tile([128, 1024], F32, tag="st")
                        nc.tensor.matmul(
                            st[:, 0:512],
                            lhsT=kT[0:64, kc * 128:(kc + 1) * 128],
                            rhs=qT[0:64, qb * 512:(qb + 1) * 512],
                            start=True, stop=True)
                        nc.tensor.matmul(
                            st[:, 512:1024],
                            lhsT=kT[64:128, kc * 128:(kc + 1) * 128],
                            rhs=qT[64:128, qb * 512:(qb + 1) * 512],
                            start=True, stop=True, tile_position=(64, 0))
                        pT = pTp.tile([128, 1024], F16, tag="pT")
                        nc.scalar.activation(pT[:], st[:], AF.Exp)
                        with nc.allow_low_precision(
                                reason="fp16 softmax partial-sum accumulate"):
                            if kc == 1:
                                nc.vector.tensor_add(
                                    acc[:], prev_pT[:], pT[:])
                            elif kc > 1:
                                nc.vector.tensor_add(acc[:], acc[:], pT[:])
                        prev_pT = pT
                        nc.tensor.matmul(
                            pv[0:64, :],
                            lhsT=vv[:, kc * EMBED + hp * 128:
                                    kc * EMBED + hp * 128 + 64],
                            rhs=pT[:, 0:512],
                            start=(kc == 0), stop=(kc == NKC - 1))
                        nc.tensor.matmul(
                            pv[64:128, :],
                            lhsT=vv[:, kc * EMBED + hp * 128 + 64:
                                    kc * EMBED + hp * 128 + 128],
                            rhs=pT[:, 512:1024],
                            start=(kc == 0), stop=(kc == NKC - 1),
                            tile_position=(0, 64))

                    # softmax denominators: ones^T @ acc -> [1, 512] per head
                    sums = ps_sm.tile([128, 512], F32, tag="small")
                    nc.tensor.matmul(sums[0:1, :], lhsT=ones[:],
                                     rhs=acc[:, 0:512], start=True, stop=True)
                    nc.tensor.matmul(sums[32:33, :], lhsT=ones[:],
                                     rhs=acc[:, 512:1024], start=True,
                                     stop=True, tile_position=(0, 32))
                    # copy the two sum-rows (partitions 0 and 32) to SBUF in
                    # one strided DVE op, broadcast raw sums to 128
                    # partitions with a rank-1 matmul, then one fast
                    # reciprocal over the broadcast tile
                    sums_sb = misc.tile([1, 1024], F16, tag="sums_sb")
                    with nc.allow_low_precision(
                            reason="softmax denominators, fp16 ample"):
                        nc.vector.tensor_copy(
                            sums_sb[:, 0:512], sums[0:1, 0:512])
                        nc.vector.tensor_copy(
                            sums_sb[:, 512:1024], sums[32:33, 0:512])
                    bc = ps_sm.tile([128, 512], F32, tag="small")
                    nc.tensor.matmul(bc[:], lhsT=sel[:, 0:128],
                                     rhs=sums_sb[:, 0:512], start=True,
                                     stop=False)
                    nc.tensor.matmul(bc[:], lhsT=sel[:, 128:256],
                                     rhs=sums_sb[:, 512:1024], start=False,
                                     stop=True)
                    bc_sb = misc.tile([128, 512], F32, tag="bc_sb")
                    nc.vector.reciprocal_approx_fast(bc_sb[:], bc[:])
                    nc.vector.tensor_mul(
                        aT[:, hp * TQ + qb * 512: hp * TQ + qb * 512 + 512],
                        pv[:], bc_sb[:])

            # ---- remaining out projection (qb1 token blocks) ----
            for tb in range(TQ // 256, TQ // 128):
                for ob in range(2):
                    o_proj_unit(tb, ob)

    nc.compile()
    return nc


def _get_program():
    global _PROGRAM
    if _PROGRAM is None:
        _PROGRAM = _build_program()
    return _PROGRAM


def _make_in_maps(x, q_w, q_b, k_w, k_b, v_w, v_b, o_w, o_b):
    f16 = np.float16
    # softmax scale folded into the Q projection
    wqT = np.ascontiguousarray((q_w.astype(np.float32).T / 8.0)).astype(f16)
    wkT = np.ascontiguousarray(k_w.astype(np.float32).T).astype(f16)
    wvT = np.ascontiguousarray(v_w.astype(np.float32).T).astype(f16)
    woT = np.ascontiguousarray(o_w.astype(np.float32).T).astype(f16)
    qb = np.ascontiguousarray(
        (q_b.astype(np.float32) / 8.0).reshape(NIC, 128).T)
    kb = np.ascontiguousarray(k_b.astype(np.float32).reshape(NIC, 128).T)
    vbb = np.broadcast_to(v_b.astype(np.float32), (128, EMBED)).astype(f16)
    vbb = np.ascontiguousarray(vbb)
    obb = np.ascontiguousarray(
        np.broadcast_to(o_b.astype(np.float32), (128, EMBED)))
    ones = np.ones((128, 1), f16)
    sel = np.zeros((1, 256), f16)
    sel[0, 0:64] = 1.0
    sel[0, 192:256] = 1.0
    in_maps = []
    for c in range(NCORES):
        b, qh = c // 2, c % 2
        xb = x[b].astype(np.float32)  # [T, EMBED]
        if qh == 0:
            xp = xb
        else:
            # query half first; K/V order is irrelevant (softmax sums over k)
            xp = np.concatenate([xb[TQ:], xb[:TQ]], axis=0)
        xT = np.ascontiguousarray(xp.T).astype(f16)
        in_maps.append({
            "xT": xT, "wqT": wqT, "wkT": wkT, "wvT": wvT, "woT": woT,
            "qb": qb, "kb": kb, "vbb": vbb, "obb": obb,
            "ones": ones, "sel": sel,
        })
    return in_maps


def kernel(x, mask, q_w, q_b, k_w, k_b, v_w, v_b, o_w, o_b):
    from concourse.bass_utils import run_bass_kernel_spmd

    nc = _get_program()
    x = np.asarray(x)
    in_maps = _make_in_maps(np.asarray(x), np.asarray(q_w), np.asarray(q_b),
                            np.asarray(k_w), np.asarray(k_b),
                            np.asarray(v_w), np.asarray(v_b),
                            np.asarray(o_w), np.asarray(o_b))
    res = run_bass_kernel_spmd(nc, in_maps, list(range(NCORES)))
    out = np.empty((B, T, EMBED), np.float32)
    for c in range(NCORES):
        b, qh = c // 2, c % 2
        out[b, qh * TQ:(qh + 1) * TQ, :] = res.results[c]["y"]
    return out
